# revision 2
# baseline (speedup 1.0000x reference)
"""Complex self-attention (BN -> complex 1x1 QKV -> real-part attention ->
softmax -> AV -> residual) on 8 Trainium2 NeuronCores.

Sharding: data-parallel over batch (B=2) x query-sequence shard (N/4 = 1024
query rows) -> 8 shards.  Each core computes BN stats (over both batches),
folds BN into the QKV weights, computes K (and V^T directly) for its whole
batch, Q for its query quarter, then flash-style attention with the (N,N)
score map never leaving SBUF/PSUM.
"""

import numpy as np

import concourse.bass as bass
import concourse.tile as tile
from concourse import bacc, mybir
from concourse.bass_utils import run_bass_kernel_spmd
from concourse.masks import make_identity

B, C, HH, WW = 2, 128, 64, 64
N = HH * WW                  # 4096 tokens
NCORES = 8
QSH = 4                      # query shards per batch
NQ = N // QSH                # 1024 query rows per core
EPS = 1e-5
SM_SCALE = float(C) ** -0.5

F32 = mybir.dt.float32
MM_DT = mybir.dt.float32     # matmul operand dtype (flip to bfloat16 later)
MMF = 512 if MM_DT == F32 else 1024   # max moving-operand free size

QT = 128                     # query-tile rows (partition dim of scores)
NQT = NQ // QT               # 8 query tiles per core
PAIR = 2                     # query tiles processed together in AV
NPAIR = NQT // PAIR


def _build(gamma: float):
    nc = bacc.Bacc("TRN2", target_bir_lowering=False, debug=False,
                   num_devices=NCORES)

    x_own = nc.dram_tensor("x_own", [C, N, 2], F32, kind="ExternalInput").ap()
    x_oth = nc.dram_tensor("x_oth", [C, N, 2], F32, kind="ExternalInput").ap()
    xq_d = nc.dram_tensor("xq", [C, NQ, 2], F32, kind="ExternalInput").ap()
    w_d = {p: nc.dram_tensor(f"w{p}", [C, C, 2], F32, kind="ExternalInput").ap()
           for p in "qkv"}
    b_d = {p: nc.dram_tensor(f"b{p}", [C, 2], F32, kind="ExternalInput").ap()
           for p in "qkv"}
    bnw_d = nc.dram_tensor("bn_w", [C, 2], F32, kind="ExternalInput").ap()
    bnb_d = nc.dram_tensor("bn_b", [C, 2], F32, kind="ExternalInput").ap()
    out_d = nc.dram_tensor("out", [C, NQ, 2], F32, kind="ExternalOutput").ap()

    with tile.TileContext(nc) as tc:
        _emit(tc, nc, gamma, x_own, x_oth, xq_d, w_d, b_d, bnw_d, bnb_d, out_d)
    nc.compile()
    return nc


def _emit(tc, nc, gamma, x_own, x_oth, xq_d, w_d, b_d, bnw_d, bnb_d, out_d):
    with tc.tile_pool(name="consts", bufs=1) as consts, \
         tc.tile_pool(name="persist", bufs=1) as persist:

        # ---- constant loads -------------------------------------------------
        wT = {}
        for p in "qkv":
            for t, tag in ((0, "r"), (1, "i")):
                wt = consts.tile([C, C], F32, name=f"wT_{p}{tag}")
                nc.sync.dma_start(out=wt, in_=w_d[p][:, :, t].rearrange("o c -> c o"))
                wT[p, tag] = wt
        bvec = {}
        for p in "qkv":
            bt = consts.tile([C, 2], F32, name=f"b_{p}")
            nc.sync.dma_start(out=bt, in_=b_d[p])
            bvec[p] = bt
        bnw = consts.tile([C, 2], F32, name="bnw")
        nc.sync.dma_start(out=bnw, in_=bnw_d)
        bnb = consts.tile([C, 2], F32, name="bnb")
        nc.sync.dma_start(out=bnb, in_=bnb_d)
        ident = consts.tile([C, C], F32, name="ident")
        make_identity(nc, ident)

        # ---- persistent operand tiles --------------------------------------
        K = {t: persist.tile([C, N], MM_DT, name=f"K_{t}") for t in "ri"}
        VT = {t: persist.tile([C, N], MM_DT, name=f"VT_{t}") for t in "ri"}
        Q = {t: persist.tile([C, NQ], MM_DT, name=f"Q_{t}") for t in "ri"}
        xq = {t: persist.tile([C, NQ], F32, name=f"xq_{t}") for t in "ri"}
        if MM_DT == F32:
            xq_mm = xq
        else:
            xq_mm = {t: persist.tile([C, NQ], MM_DT, name=f"xqmm_{t}")
                     for t in "ri"}

        NCH = 8                     # 512-token chunks per batch
        CH = N // NCH
        stats = {t: consts.tile([C, 2 * NCH, 6], F32, name=f"stats_{t}")
                 for t in "ri"}

        with tc.tile_pool(name="xsplit", bufs=1) as xsplit:
            xr = {t: xsplit.tile([C, N], MM_DT, name=f"x_{t}") for t in "ri"}

            # ---- phase 1: load x, BN stats, de-interleave ------------------
            with tc.tile_pool(name="ld", bufs=3) as ld:
                for j in range(NCH):
                    chunk = ld.tile([C, CH, 2], F32, tag="chunk")
                    nc.sync.dma_start(out=chunk,
                                      in_=x_own[:, j * CH:(j + 1) * CH, :])
                    for ti, t in ((0, "r"), (1, "i")):
                        nc.vector.bn_stats(out=stats[t][:, j, :],
                                           in_=chunk[:, :, ti])
                        nc.vector.tensor_copy(
                            out=xr[t][:, j * CH:(j + 1) * CH],
                            in_=chunk[:, :, ti])
                for j in range(NCH):
                    chunk = ld.tile([C, CH, 2], F32, tag="chunk")
                    nc.sync.dma_start(out=chunk,
                                      in_=x_oth[:, j * CH:(j + 1) * CH, :])
                    for ti, t in ((0, "r"), (1, "i")):
                        nc.vector.bn_stats(out=stats[t][:, NCH + j, :],
                                           in_=chunk[:, :, ti])
                for j in range(2):
                    chunk = ld.tile([C, CH, 2], F32, tag="chunk")
                    nc.sync.dma_start(out=chunk,
                                      in_=xq_d[:, j * CH:(j + 1) * CH, :])
                    for ti, t in ((0, "r"), (1, "i")):
                        nc.vector.tensor_copy(
                            out=xq[t][:, j * CH:(j + 1) * CH],
                            in_=chunk[:, :, ti])
                        if MM_DT != F32:
                            nc.vector.tensor_copy(
                                out=xq_mm[t][:, j * CH:(j + 1) * CH],
                                in_=chunk[:, :, ti])

            # ---- BN scale/offset -------------------------------------------
            s, o = {}, {}
            for ti, t in ((0, "r"), (1, "i")):
                mv = consts.tile([C, 2], F32, name=f"mv_{t}")
                nc.vector.bn_aggr(out=mv, in_=stats[t])
                veps = consts.tile([C, 1], F32, name=f"veps_{t}")
                nc.vector.tensor_scalar_add(veps, mv[:, 1:2], EPS)
                std = consts.tile([C, 1], F32, name=f"std_{t}")
                nc.scalar.sqrt(std, veps)
                rstd = consts.tile([C, 1], F32, name=f"rstd_{t}")
                nc.vector.reciprocal(rstd, std)
                st = consts.tile([C, 1], F32, name=f"s_{t}")
                nc.vector.tensor_mul(st, rstd, bnw[:, ti:ti + 1])
                ot = consts.tile([C, 1], F32, name=f"o_{t}")
                ms = consts.tile([C, 1], F32, name=f"ms_{t}")
                nc.vector.tensor_mul(ms, mv[:, 0:1], st)
                nc.vector.tensor_sub(ot, bnb[:, ti:ti + 1], ms)
                s[t], o[t] = st, ot
            neg_si = consts.tile([C, 1], F32, name="neg_si")
            nc.vector.tensor_scalar_mul(neg_si, s["i"], -1.0)

            # ---- fold BN into projection weights ---------------------------
            # Y_r = (Wr.s_r)@xr + (Wi.(-s_i))@xi + beta_r
            # Y_i = (Wi.s_r)@xr + (Wr.s_i)@xi + beta_i
            WA, WB, WC, WD, beta = {}, {}, {}, {}, {}
            for p in "qkv":
                WA[p] = consts.tile([C, C], MM_DT, name=f"WA_{p}")
                nc.vector.tensor_scalar_mul(WA[p], wT[p, "r"], s["r"])
                WB[p] = consts.tile([C, C], MM_DT, name=f"WB_{p}")
                nc.vector.tensor_scalar_mul(WB[p], wT[p, "i"], neg_si)
                WC[p] = consts.tile([C, C], MM_DT, name=f"WC_{p}")
                nc.vector.tensor_scalar_mul(WC[p], wT[p, "i"], s["r"])
                WD[p] = consts.tile([C, C], MM_DT, name=f"WD_{p}")
                nc.vector.tensor_scalar_mul(WD[p], wT[p, "r"], s["i"])

            # beta_p = [Wr@o_r - Wi@o_i | Wi@o_r + Wr@o_i] + b_p
            or_oi = consts.tile([C, 2], F32, name="or_oi")
            nc.vector.tensor_copy(or_oi[:, 0:1], o["r"])
            nc.vector.tensor_copy(or_oi[:, 1:2], o["i"])
            noi_or = consts.tile([C, 2], F32, name="noi_or")
            nc.vector.tensor_scalar_mul(noi_or[:, 0:1], o["i"], -1.0)
            nc.vector.tensor_copy(noi_or[:, 1:2], o["r"])
            with tc.tile_pool(name="bias_ps", bufs=1, space="PSUM") as bps:
                for p in "qkv":
                    ps = bps.tile([C, 2], F32, tag="bias")
                    nc.tensor.matmul(ps, wT[p, "r"], or_oi, start=True, stop=False)
                    nc.tensor.matmul(ps, wT[p, "i"], noi_or, start=False, stop=True)
                    bt = consts.tile([C, 2], F32, name=f"beta_{p}")
                    nc.vector.tensor_add(bt, ps, bvec[p])
                    beta[p] = bt

            # ---- projections ------------------------------------------------
            with tc.tile_pool(name="proj_ps", bufs=2, space="PSUM") as pps, \
                 tc.tile_pool(name="vt_ps", bufs=2, space="PSUM") as vps:
                QC = 1024
                for ti, t, w1, w2 in ((0, "r", WA, WB), (1, "i", WC, WD)):
                    bsl = beta["k"][:, ti:ti + 1]
                    for q4 in range(N // QC):
                        ps = pps.tile([C, QC], F32, tag="proj")
                        for si, wm in ((True, w1), (False, w2)):
                            src = xr["r"] if si else xr["i"]
                            for j in range(QC // MMF):
                                nc.tensor.matmul(
                                    ps[:, j * MMF:(j + 1) * MMF], wm["k"],
                                    src[:, q4 * QC + j * MMF:
                                        q4 * QC + (j + 1) * MMF],
                                    start=si, stop=not si)
                        nc.vector.tensor_scalar_add(
                            K[t][:, q4 * QC:(q4 + 1) * QC], ps, bsl)
                    bsl = beta["q"][:, ti:ti + 1]
                    ps = pps.tile([C, QC], F32, tag="proj")
                    for si, wm in ((True, w1), (False, w2)):
                        src = xq_mm["r"] if si else xq_mm["i"]
                        for j in range(QC // MMF):
                            nc.tensor.matmul(
                                ps[:, j * MMF:(j + 1) * MMF], wm["q"],
                                src[:, j * MMF:(j + 1) * MMF],
                                start=si, stop=not si)
                    nc.vector.tensor_scalar_add(Q[t], ps, bsl)

                # V^T: for each 128-token chunk ch, out[m,o] = xn_ch.T @ W
                for grp in range(8):
                    ps = vps.tile([C, 2, 512], F32, tag="vt")
                    for k4 in range(4):
                        ch = grp * 4 + k4
                        xr_ch = xr["r"][:, ch * 128:(ch + 1) * 128]
                        xi_ch = xr["i"][:, ch * 128:(ch + 1) * 128]
                        sl = ps[:, 0, k4 * 128:(k4 + 1) * 128]
                        si_ = ps[:, 1, k4 * 128:(k4 + 1) * 128]
                        nc.tensor.matmul(sl, xr_ch, WA["v"], start=True, stop=False)
                        nc.tensor.matmul(si_, xr_ch, WC["v"], start=True, stop=False)
                        nc.tensor.matmul(sl, xi_ch, WB["v"], start=False, stop=True)
                        nc.tensor.matmul(si_, xi_ch, WD["v"], start=False, stop=True)
                    nc.vector.tensor_copy(VT["r"][:, grp * 512:(grp + 1) * 512],
                                          ps[:, 0, :])
                    nc.vector.tensor_copy(VT["i"][:, grp * 512:(grp + 1) * 512],
                                          ps[:, 1, :])
        # xsplit pool (xr/xi) freed here.

        # residual constants: oo_t = o_t + gamma * beta_v_t
        oo = {}
        for ti, t in ((0, "r"), (1, "i")):
            gb = consts.tile([C, 1], F32, name=f"gbv_{t}")
            nc.vector.tensor_scalar_mul(gb, beta["v"][:, ti:ti + 1], float(gamma))
            oot = consts.tile([C, 1], F32, name=f"oo_{t}")
            nc.vector.tensor_add(oot, o[t], gb)
            oo[t] = oot

        # ---- attention ------------------------------------------------------
        with tc.tile_pool(name="attn", bufs=1) as attn, \
             tc.tile_pool(name="sm", bufs=4) as sm, \
             tc.tile_pool(name="fin", bufs=2) as finp, \
             tc.tile_pool(name="s_ps", bufs=2, space="PSUM") as sps, \
             tc.tile_pool(name="t_ps", bufs=2, space="PSUM") as tps, \
             tc.tile_pool(name="o_ps", bufs=1, space="PSUM") as ops:
            for pair in range(NPAIR):
                A2 = attn.tile([QT, PAIR, N], MM_DT, tag="A2")
                AT2 = attn.tile([C, PAIR, N], MM_DT, tag="AT2")
                for sub in range(PAIR):
                    qt = pair * PAIR + sub
                    qoff = qt * QT
                    ssum = sm.tile([QT, 4], F32, tag="ssum")
                    for quarter in range(4):
                        ps = sps.tile([QT, 1024], F32, tag="s")
                        for ci, t in ((0, "r"), (1, "i")):
                            for j in range(1024 // MMF):
                                nc.tensor.matmul(
                                    ps[:, j * MMF:(j + 1) * MMF],
                                    Q[t][:, qoff:qoff + QT],
                                    K[t][:, quarter * 1024 + j * MMF:
                                         quarter * 1024 + (j + 1) * MMF],
                                    start=(ci == 0), stop=(ci == 1))
                        nc.scalar.activation(
                            out=A2[:, sub, quarter * 1024:(quarter + 1) * 1024],
                            in_=ps, func=mybir.ActivationFunctionType.Exp,
                            scale=SM_SCALE,
                            accum_out=ssum[:, quarter:quarter + 1])
                    dsum = sm.tile([QT, 1], F32, tag="dsum")
                    nc.vector.reduce_sum(dsum, ssum, axis=mybir.AxisListType.X)
                    rd = sm.tile([QT, 1], F32, tag="rd")
                    nc.vector.reciprocal(rd, dsum)
                    diag = sm.tile([QT, QT], MM_DT, tag="diag")
                    nc.vector.tensor_scalar_mul(diag, ident, rd)
                    # transpose A (folding in 1/rowsum): AT[m,n] = A[n,m]*rd[n]
                    for grp in range(8):
                        ps = tps.tile([C, 512], F32, tag="t")
                        for k4 in range(4):
                            ch = grp * 4 + k4
                            nc.tensor.matmul(
                                ps[:, k4 * 128:(k4 + 1) * 128],
                                A2[:, sub, ch * 128:(ch + 1) * 128],
                                diag, start=True, stop=True)
                        nc.vector.tensor_copy(
                            AT2[:, sub, grp * 512:(grp + 1) * 512], ps)
                # AV: out[c, (sub, nq)] accumulated over 32 m-chunks
                po = ops.tile([C, 2, 512], F32, tag="o")
                for ch in range(32):
                    at_sl = AT2[:, :, ch * 128:(ch + 1) * 128]
                    for ti, t in ((0, "r"), (1, "i")):
                        nc.tensor.matmul(
                            po[:, ti, 0:PAIR * QT],
                            VT[t][:, ch * 128:(ch + 1) * 128],
                            at_sl, start=(ch == 0), stop=(ch == 31))
                # residual + interleave + store
                fin = finp.tile([C, PAIR * QT, 2], F32, tag="fin")
                for sub in range(PAIR):
                    for ti, t in ((0, "r"), (1, "i")):
                        xn = sm.tile([C, QT], F32, tag="xn")
                        nc.vector.tensor_scalar(
                            out=xn,
                            in0=xq[t][:, (pair * PAIR + sub) * QT:
                                      (pair * PAIR + sub + 1) * QT],
                            scalar1=s[t], scalar2=oo[t],
                            op0=mybir.AluOpType.mult, op1=mybir.AluOpType.add)
                        nc.vector.scalar_tensor_tensor(
                            out=fin[:, sub * QT:(sub + 1) * QT, ti],
                            in0=po[:, ti, sub * QT:(sub + 1) * QT],
                            scalar=float(gamma), in1=xn,
                            op0=mybir.AluOpType.mult, op1=mybir.AluOpType.add)
                nc.sync.dma_start(
                    out=out_d[:, pair * PAIR * QT:(pair + 1) * PAIR * QT, :],
                    in_=fin)


_CACHE = {}


def _get_nc(gamma: float):
    key = float(gamma)
    if key not in _CACHE:
        _CACHE[key] = _build(key)
    return _CACHE[key]


def kernel(x, bn_w, bn_b, wq, bq, wk, bk, wv, bv, gamma):
    x = np.asarray(x, dtype=np.float32)
    nc = _get_nc(float(gamma))
    xt = x.reshape(B, C, N, 2)
    in_maps = []
    for core in range(NCORES):
        b, qs = divmod(core, QSH)
        in_maps.append({
            "x_own": xt[b],
            "x_oth": xt[1 - b],
            "xq": np.ascontiguousarray(xt[b][:, qs * NQ:(qs + 1) * NQ, :]),
            "bn_w": np.asarray(bn_w, np.float32),
            "bn_b": np.asarray(bn_b, np.float32),
            "wq": np.asarray(wq, np.float32), "bq": np.asarray(bq, np.float32),
            "wk": np.asarray(wk, np.float32), "bk": np.asarray(bk, np.float32),
            "wv": np.asarray(wv, np.float32), "bv": np.asarray(bv, np.float32),
        })
    res = run_bass_kernel_spmd(nc, in_maps, list(range(NCORES)))
    out = np.empty((B, C, N, 2), np.float32)
    for core in range(NCORES):
        b, qs = divmod(core, QSH)
        out[b][:, qs * NQ:(qs + 1) * NQ, :] = res.results[core]["out"]
    return out.reshape(B, C, HH, WW, 2)
